# revision 31
# baseline (speedup 1.0000x reference)
"""Trainium2 Bass kernel for BaseBidirectionalAttention.

Problem shapes (hardcoded): B=32, C=1024, Q=128, D=256, F=4D=1024.
Sharding: data-parallel over batch across 8 cores (4 batch elems/core);
weights replicated.

Per-core program (per batch elem), all in fp32:
  sim(C,Q)   = ctx @ (q*wm).T + cwc[:,None] + qwq[None,:]   (PE, c on partitions)
  P          = softmax_q(sim)                                (DVE/ACT, free-dim)
  c2q^T(D,C) = question.T @ P.T                              (PE, via P transpose)
  q2c(D)     = softmax_c(max_q sim) @ ctx                    (PE + transpose trick)
  att^T      = [ctx^T; c2q^T; ctx^T*c2q^T; ctx^T*q2c]        (f on partitions)
  h1^T(F,C)  = W1 @ att^T + b1                               (PE, lhsT=W1T tiles)
  out(C,F)   = relu((h1 @ W2.T + b2)) * mask                 (PE, natural layout)
"""

import sys

if "/opt/trn_rl_repo" not in sys.path:
    sys.path.insert(0, "/opt/trn_rl_repo")

import numpy as np

import concourse.bass as bass
import concourse.mybir as mybir
import concourse.tile as tile
from concourse import bacc
from concourse.bass_utils import run_bass_kernel_spmd
from concourse.masks import make_identity

B, C, Q, D = 32, 1024, 128, 256
F = 4 * D
NCORES = 8
BPC = B // NCORES  # batch elems per core
P = 128
CT = C // P   # 8 c-tiles
FT = F // P   # 8 f-tiles
DH = D // P   # 2 halves of D
NH = C // 512  # 2 c-chunks of 512
FH = F // 512  # 2 f'-chunks of 512

FP32 = mybir.dt.float32
FP32R = mybir.dt.float32r
AX = mybir.AxisListType.X
AF = mybir.ActivationFunctionType


def _r(ap):
    """Bitcast an fp32 AP to float32r for full-rate PE streaming."""
    return ap.bitcast(FP32R)


def _f(ap):
    """fp32 view of a float32r AP (same bits) for DVE/fp32-matmul reads."""
    return ap.bitcast(FP32)


def _build_body(es, tc, outs, ins, n_elems=BPC, reps=1):
    nc = tc.nc
    ctx_d, qst_d, vecsT_d, w1t_d, w2t_d, b1c_d, b2r_d, mT_d = ins
    out_d = outs[0]

    const = es.enter_context(tc.tile_pool(name="const", bufs=1))
    weights = es.enter_context(tc.tile_pool(name="weights", bufs=1))
    loads = es.enter_context(tc.tile_pool(name="loads", bufs=2))
    work = es.enter_context(tc.tile_pool(name="work", bufs=1))
    outp = es.enter_context(tc.tile_pool(name="outp", bufs=3))
    psA = es.enter_context(tc.tile_pool(name="psA", bufs=5, space="PSUM"))
    psB = es.enter_context(tc.tile_pool(name="psB", bufs=3, space="PSUM"))

    # ---- constants / replicated weights ----
    ident = const.tile([P, P], FP32)
    make_identity(nc, ident)
    ones_row = const.tile([1, P], FP32)
    nc.vector.memset(ones_row, 1.0)
    ones_col = const.tile([P, 1], FP32)
    nc.vector.memset(ones_col, 1.0)

    def load_elem(b, idx):
        cn = loads.tile([P, CT, D], FP32, tag="ctx_nat", name=f"ctx_nat{idx}")
        src_ap = ctx_d[b].rearrange("(t p) d -> p t d", p=P)
        half = CT // 2
        nc.sync.dma_start(cn[:, :half], src_ap[:, :half])
        nc.sync.dma_start(cn[:, half:], src_ap[:, half:])
        qn = loads.tile([P, D], FP32R, tag="qst_nat", name=f"qst_nat{idx}")
        nc.sync.dma_start(qn[:], qst_d[b])
        return cn, qn

    # elem-0 loads go before the big weight DMAs (single-shot only: with a
    # For_i timing loop the hoisted tile's slot would be recycled in-loop)
    pend = load_elem(0, 0) if reps == 1 else None

    vecsT = const.tile([P, DH, 3], FP32)  # [p, h, v]: wq/wc/wm at e=h*128+p
    nc.sync.dma_start(vecsT[:], vecsT_d.rearrange("(h p) v -> p h v", p=P))

    w1t = weights.tile([P, FT, F], FP32R)  # [fl, k, f'] = W1[f', k*128+fl]
    nc.sync.dma_start(w1t[:], w1t_d.rearrange("(k p) f -> p k f", p=P))
    w2t = weights.tile([P, FT, F], FP32R)  # [fl, k, f'] = W2[f', k*128+fl]
    nc.sync.dma_start(w2t[:], w2t_d.rearrange("(k p) f -> p k f", p=P))
    b1c = const.tile([P, FT], FP32)  # [p, mf] = b1[mf*128+p]
    nc.sync.dma_start(b1c[:], b1c_d)
    b2bc = const.tile([P, F], FP32)  # b2 broadcast to all partitions
    b2r_ap = b2r_d  # (1, F) in dram
    nc.gpsimd.dma_start(
        out=b2bc[:],
        in_=bass.AP(tensor=b2r_ap.tensor, offset=b2r_ap.offset, ap=[[0, P]] + b2r_ap.ap[1:]),
    )
    mT = const.tile([P, n_elems * CT], FP32)  # [p, b*8+t] = mask[b, t*128+p]
    nc.sync.dma_start(mT[:], mT_d)

    if reps > 1:
        es.enter_context(tc.For_i(0, reps, 1))

    def preamble(ctx_nat, qst_nat, idx):
        """PE transposes + DVE copies producing ctx^T, q^T, (q*wm)^T, qwq."""
        ctxT = work.tile([P, DH, C], FP32R, tag="ctxT", bufs=2, name=f"ctxT{idx}")
        for dh in range(DH):
            for g in range(2):  # two groups of 4 c-tiles -> one psum bank each
                pt = psA.tile([P, 512], FP32, tag="ps_mm", name=f"ptc{idx}{dh}{g}")
                for j in range(4):
                    t = g * 4 + j
                    nc.tensor.transpose(
                        pt[:, j * P:(j + 1) * P],
                        ctx_nat[:, t, dh * P:(dh + 1) * P],
                        ident[:],
                    )
                nc.vector.tensor_copy(ctxT[:, dh, g * 512:(g + 1) * 512], pt[:])

        qstT = work.tile([P, DH, P], FP32, tag="qstT", bufs=2, name=f"qstT{idx}")
        qmT = work.tile([P, DH, P], FP32R, tag="qmT", bufs=2, name=f"qmT{idx}")
        pq = psB.tile([P, 2 * P], FP32, tag="ps_small", name=f"pq{idx}")
        for dh in range(DH):
            nc.tensor.transpose(pq[:, dh * P:(dh + 1) * P],
                                _f(qst_nat[:, dh * P:(dh + 1) * P]), ident[:])
        nc.vector.tensor_copy(qstT[:].rearrange("p h q -> p (h q)"), pq[:])
        for dh in range(DH):
            nc.vector.tensor_scalar_mul(qmT[:, dh, :], qstT[:, dh, :], vecsT[:, dh, 2:3])

        # qwq term; cwc cancels in softmax-q entirely.  qwq needs no
        # per-sim-tile matmul: a broadcast tile is added during the fused
        # negate+max reduce, and exp reads the negated full logits.
        qwq = work.tile([1, P], FP32, tag="qwq", bufs=2, name=f"qwq{idx}")
        pw = psB.tile([1, P], FP32, tag="ps_small", name=f"pw{idx}")
        for dh in range(DH):
            nc.tensor.matmul(
                pw[:], vecsT[:, dh, 0:1], qstT[:, dh, :],
                start=(dh == 0), stop=(dh == DH - 1),
            )
        nc.vector.tensor_copy(qwq[:], pw[:])
        pqb = psA.tile([P, P], FP32, tag="ps_mm", name=f"pqb{idx}")
        nc.tensor.matmul(pqb[:], ones_row[:], qwq[:], start=True, stop=True)
        qwqbc = work.tile([P, P], FP32, tag="qwqbc", bufs=2, name=f"qwqbc{idx}")
        nc.vector.tensor_copy(qwqbc[:], pqb[:])
        return ctxT, qstT, qmT, qwqbc

    pre = None
    for b in range(n_elems):
        # ---- loads (elem b prefetched; prefetch b+1 now) ----
        if pend is None:
            pend = load_elem(b, b)
        ctx_nat, qst_nat = pend
        pend = load_elem(b + 1, b + 1) if b + 1 < n_elems else None
        if pre is None:
            pre = preamble(ctx_nat, qst_nat, b)
        ctxT, qstT, qmT, qwqbc = pre

        # ---- sim tiles + softmax over q (free dim) ----
        nmx = work.tile([P, CT], FP32, tag="nmx")     # negated row max per c-tile
        Pm = work.tile([P, CT, P], FP32, tag="Pm")    # softmax(sim), [c_l, t, q]
        sume = work.tile([P, CT], FP32, tag="sume")
        rs = work.tile([P, CT], FP32, tag="rs")
        for t in range(CT):
            ps = psB.tile([P, P], FP32, tag="ps_small")
            for dh in range(DH):
                nc.tensor.matmul(
                    ps[:], ctxT[:, dh, t * P:(t + 1) * P], qmT[:, dh, :],
                    start=(dh == 0), stop=(dh == DH - 1),
                )
            # scr = base + qwq (full logits); nmx = -max_q; exp with
            # fused row-sum accumulate
            scr = work.tile([P, P], FP32, tag="scr", bufs=2)
            nc.vector.tensor_add(scr[:], ps[:], qwqbc[:])
            nc.vector.reduce_max(nmx[:, t:t + 1], scr[:], axis=AX, negate=True)
            nc.scalar.activation(
                Pm[:, t, :], scr[:], AF.Exp, bias=nmx[:, t:t + 1],
                accum_out=sume[:, t:t + 1],
            )
            nc.vector.reciprocal(rs[:, t:t + 1], sume[:, t:t + 1])
            nc.vector.tensor_scalar_mul(Pm[:, t, :], Pm[:, t, :], rs[:, t:t + 1])
        PT = work.tile([P, C], FP32R, tag="PT")  # [q, c]
        for g in range(2):
            pt = psA.tile([P, 512], FP32, tag="ps_mm")
            for j in range(4):
                t = g * 4 + j
                nc.tensor.transpose(pt[:, j * P:(j + 1) * P], Pm[:, t, :], ident[:])
            nc.vector.tensor_copy(PT[:, g * 512:(g + 1) * 512], pt[:])

        # ---- q2c weights: softmax over all C of (max_q sim + cwc) ----
        pcw = psB.tile([P, CT], FP32, tag="ps_small")  # cwc as columns [c_l, t]
        for t in range(CT):
            for dh in range(DH):
                nc.tensor.matmul(
                    pcw[:, t:t + 1], _f(ctxT[:, dh, t * P:(t + 1) * P]),
                    vecsT[:, dh, 1:2],
                    start=(dh == 0), stop=(dh == DH - 1),
                )
        madj = work.tile([P, CT], FP32, tag="madj")  # m_c = cwc - nmx
        nc.vector.tensor_sub(madj[:], pcw[:], nmx[:])
        colmin = work.tile([P, 1], FP32, tag="colmin")
        nc.vector.reduce_max(colmin[:], madj[:], axis=AX, negate=True)
        pcm = psB.tile([1, P], FP32, tag="ps_small")
        nc.tensor.transpose(pcm[:], colmin[:], ident[:])
        minall = work.tile([1, 2], FP32, tag="minall")
        nc.vector.tensor_reduce(minall[:, 0:1], pcm[:], axis=AX, op=mybir.AluOpType.min)
        pmb = psB.tile([P, 1], FP32, tag="ps_small")
        nc.tensor.matmul(pmb[:], ones_row[:], minall[:, 0:1], start=True, stop=True)
        minb = work.tile([P, 1], FP32, tag="minb")
        nc.vector.tensor_copy(minb[:], pmb[:])
        wall = work.tile([P, CT], FP32, tag="wall")  # exp(m - Mglob)
        nc.scalar.activation(wall[:], madj[:], AF.Exp, bias=minb[:])

        # numerator columns (d,1) x2 and denominator
        pnum = [psB.tile([P, 1], FP32, tag="ps_small", name=f"pnum{dh}")
                for dh in range(DH)]
        for dh in range(DH):
            for t in range(CT):
                nc.tensor.matmul(
                    pnum[dh][:], ctx_nat[:, t, dh * P:(dh + 1) * P], wall[:, t:t + 1],
                    start=(t == 0), stop=(t == CT - 1),
                )
        pden = psB.tile([1, 1], FP32, tag="ps_small")
        for t in range(CT):
            nc.tensor.matmul(
                pden[:], wall[:, t:t + 1], ones_col[:],
                start=(t == 0), stop=(t == CT - 1),
            )
        rden = work.tile([1, 1], FP32, tag="rden")
        nc.vector.reciprocal(rden[:], pden[:])
        prb = psB.tile([P, 1], FP32, tag="ps_small")
        nc.tensor.matmul(prb[:], ones_row[:], rden[:], start=True, stop=True)
        rdenb = work.tile([P, 1], FP32, tag="rdenb")
        nc.vector.tensor_copy(rdenb[:], prb[:])
        q2c = work.tile([P, DH], FP32, tag="q2c")  # [d_l, dh]
        for dh in range(DH):
            nc.vector.tensor_mul(q2c[:, dh:dh + 1], pnum[dh][:], rdenb[:])

        # ---- att^T pieces ----
        c2qT = work.tile([P, DH, C], FP32R, tag="c2qT")
        for dh in range(DH):
            for g in range(NH):
                pc2 = psA.tile([P, 512], FP32, tag="ps_mm")
                nc.tensor.matmul(
                    pc2[:], qst_nat[:, dh * P:(dh + 1) * P],
                    PT[:, g * 512:(g + 1) * 512],
                    start=True, stop=True,
                )
                nc.vector.tensor_copy(c2qT[:, dh, g * 512:(g + 1) * 512], pc2[:])
        cxc = work.tile([P, DH, C], FP32R, tag="cxc")
        cxq = work.tile([P, DH, C], FP32R, tag="cxq")
        for dh in range(DH):
            nc.vector.tensor_mul(cxc[:, dh, :], _f(ctxT[:, dh, :]), _f(c2qT[:, dh, :]))
            nc.vector.tensor_scalar_mul(cxq[:, dh, :], _f(ctxT[:, dh, :]), q2c[:, dh:dh + 1])

        att_pieces = [ctxT, c2qT, cxc, cxq]  # k-tile = att_pieces[k//2][:, k%2, :]

        # ---- layer 1: h1^T[f', c] = W1 @ att^T + b1 ----
        h1T = work.tile([P, FT, C], FP32R, tag="h1T")
        for mf in range(FT):
            # both c-chunks per mf with g innermost: consecutive matmul pairs
            # share the same stationary lhsT (weight slice)
            phs = [psA.tile([P, 512], FP32, tag="ps_mm", name=f"ph{mf}{g}")
                   for g in range(NH)]
            for k in range(FT):
                for g in range(NH):
                    rhs = att_pieces[k // 2][:, k % 2, g * 512:(g + 1) * 512]
                    nc.tensor.matmul(
                        phs[g][:], w1t[:, k, mf * P:(mf + 1) * P], rhs,
                        start=(k == 0), stop=(k == FT - 1),
                    )
            for g in range(NH):
                # alternate PSUM eviction between ACT and DVE to release
                # psA slots faster
                if (mf * NH + g) % 2 == 0:
                    nc.scalar.add(h1T[:, mf, g * 512:(g + 1) * 512], phs[g][:],
                                  b1c[:, mf:mf + 1])
                else:
                    nc.vector.tensor_scalar_add(h1T[:, mf, g * 512:(g + 1) * 512],
                                                phs[g][:], b1c[:, mf:mf + 1])

        # next elem's transpose preamble: PE runs it here so its DVE copies
        # overlap layer-2 matmuls instead of stalling at the elem boundary
        pre = preamble(pend[0], pend[1], b + 1) if pend is not None else None

        # ---- layer 2 (natural layout) + bias + mask + relu + store ----
        for ct in range(CT):
            osb = outp.tile([P, F], FP32, tag="osb")
            # both f'-chunks with fh innermost: consecutive matmul pairs share
            # the same stationary lhsT (h1^T slice)
            p2s = [psA.tile([P, 512], FP32, tag="ps_mm", name=f"p2{ct}{fh}")
                   for fh in range(FH)]
            for k in range(FT):
                for fh in range(FH):
                    nc.tensor.matmul(
                        p2s[fh][:], h1T[:, k, ct * P:(ct + 1) * P],
                        w2t[:, k, fh * 512:(fh + 1) * 512],
                        start=(k == 0), stop=(k == FT - 1),
                    )
            for fh in range(FH):
                tmp = outp.tile([P, 512], FP32, tag="tmp")
                nc.vector.tensor_add(tmp[:], p2s[fh][:],
                                     b2bc[:, fh * 512:(fh + 1) * 512])
                nc.scalar.activation(
                    osb[:, fh * 512:(fh + 1) * 512], tmp[:], AF.Relu,
                    scale=mT[:, b * CT + ct:b * CT + ct + 1],
                )
                nc.sync.dma_start(
                    out_d[b, ct * P:(ct + 1) * P, fh * 512:(fh + 1) * 512],
                    osb[:, fh * 512:(fh + 1) * 512])


_NC_CACHE = {}


def _build_nc(n_elems=BPC, reps=1):
    key = (n_elems, reps)
    if key in _NC_CACHE:
        return _NC_CACHE[key]
    nc = bacc.Bacc("TRN2", target_bir_lowering=False, debug=False, num_devices=NCORES)
    ins = [
        nc.dram_tensor("ctx", (n_elems, C, D), FP32, kind="ExternalInput").ap(),
        nc.dram_tensor("qst", (n_elems, Q, D), FP32R, kind="ExternalInput").ap(),
        nc.dram_tensor("vecsT", (D, 3), FP32, kind="ExternalInput").ap(),
        nc.dram_tensor("w1t", (F, F), FP32R, kind="ExternalInput").ap(),
        nc.dram_tensor("w2t", (F, F), FP32R, kind="ExternalInput").ap(),
        nc.dram_tensor("b1c", (P, FT), FP32, kind="ExternalInput").ap(),
        nc.dram_tensor("b2r", (1, F), FP32, kind="ExternalInput").ap(),
        nc.dram_tensor("mT", (P, n_elems * CT), FP32, kind="ExternalInput").ap(),
    ]
    outs = [nc.dram_tensor("out", (n_elems, C, F), FP32, kind="ExternalOutput").ap()]
    from contextlib import ExitStack
    with tile.TileContext(nc) as tc, ExitStack() as es:
        _build_body(es, tc, outs, ins, n_elems=n_elems, reps=reps)
    nc.compile()
    _NC_CACHE[key] = (nc, ins, outs)
    return _NC_CACHE[key]


def _host_prep(context, question, context_mask, w_question, w_context, w_multiple,
               W1, b1, W2, b2):
    """Build the 8 per-core input maps from full inputs."""
    context = np.asarray(context, np.float32)
    question = np.asarray(question, np.float32)
    maskf = np.asarray(context_mask).astype(np.float32)
    vecsT = np.ascontiguousarray(
        np.stack([w_question, w_context, w_multiple]).T.astype(np.float32))  # (D,3)
    w1t = np.ascontiguousarray(np.asarray(W1, np.float32).T)  # [f, f'] = W1[f', f]
    w2t = np.ascontiguousarray(np.asarray(W2, np.float32).T)
    b1c = np.ascontiguousarray(np.asarray(b1, np.float32).reshape(FT, P).T)  # (128, 8)
    b2r = np.asarray(b2, np.float32).reshape(1, F)
    in_maps = []
    for i in range(NCORES):
        sl = slice(BPC * i, BPC * (i + 1))
        mTc = np.ascontiguousarray(
            maskf[sl].reshape(BPC, CT, P).transpose(2, 0, 1).reshape(P, BPC * CT))
        in_maps.append({
            "ctx": np.ascontiguousarray(context[sl]),
            "qst": np.ascontiguousarray(question[sl]),
            "vecsT": vecsT,
            "w1t": w1t,
            "w2t": w2t,
            "b1c": b1c,
            "b2r": b2r,
            "mT": mTc,
        })
    return in_maps


def kernel(context, question, context_mask, w_question, w_context, w_multiple,
           W1, b1, W2, b2):
    nc, _, _ = _build_nc()
    in_maps = _host_prep(context, question, context_mask, w_question, w_context,
                         w_multiple, W1, b1, W2, b2)
    res = run_bass_kernel_spmd(nc, in_maps, list(range(NCORES))).results
    out = np.concatenate([res[i]["out"] for i in range(NCORES)], axis=0)
    return out


# revision 33
# speedup vs baseline: 1.0215x; 1.0215x over previous
"""Trainium2 Bass kernel for BaseBidirectionalAttention.

Problem shapes (hardcoded): B=32, C=1024, Q=128, D=256, F=4D=1024.
Sharding: data-parallel over batch across 8 cores (4 batch elems/core);
weights replicated.

Per-core program (per batch elem):
  sim(C,Q)   = ctx @ (q*wm).T (+qwq via broadcast add)       (PE, c on partitions)
  P          = softmax_q(sim)  [cwc term cancels here]       (DVE/ACT, free-dim)
  c2q^T(D,C) = question.T @ P.T                              (PE, via P transpose)
  q2c(D)     = softmax_c(max_q sim + cwc) @ ctx              (PE + transpose trick)
  att^T      = [ctx^T; c2q^T; ctx^T*c2q^T; ctx^T*q2c]        (f on partitions)
  h1^T(F,C)  = W1 @ att^T + b1                               (PE, lhsT=W1T tiles)
  out(C,F)   = relu((h1 @ W2.T + b2)) * mask                 (PE, natural layout)

The intermediate mask multiply of the reference is provably redundant (row
masks propagate row-wise through the linears), so masking happens once at the
end.  The heavy matmuls (sim, c2q, both linears) run in float32r: full-rate PE
streaming (~1 col/cycle vs 4 for fp32) at slightly reduced mantissa precision;
rel err vs the fp32 reference is ~5.7e-4.  Softmax statistics, exp, and all
reductions stay fp32.
"""

import sys

if "/opt/trn_rl_repo" not in sys.path:
    sys.path.insert(0, "/opt/trn_rl_repo")

import numpy as np

import concourse.bass as bass
import concourse.mybir as mybir
import concourse.tile as tile
from concourse import bacc
from concourse.bass_utils import run_bass_kernel_spmd
from concourse.masks import make_identity

B, C, Q, D = 32, 1024, 128, 256
F = 4 * D
NCORES = 8
BPC = B // NCORES  # batch elems per core
P = 128
CT = C // P   # 8 c-tiles
FT = F // P   # 8 f-tiles
DH = D // P   # 2 halves of D
NH = C // 512  # 2 c-chunks of 512
FH = F // 512  # 2 f'-chunks of 512

FP32 = mybir.dt.float32
FP32R = mybir.dt.float32r
AX = mybir.AxisListType.X
AF = mybir.ActivationFunctionType


def _r(ap):
    """Bitcast an fp32 AP to float32r for full-rate PE streaming."""
    return ap.bitcast(FP32R)


def _f(ap):
    """fp32 view of a float32r AP (same bits) for DVE/fp32-matmul reads."""
    return ap.bitcast(FP32)


def _build_body(es, tc, outs, ins, n_elems=BPC, reps=1):
    nc = tc.nc
    ctx_d, qst_d, vecsT_d, w1t_d, w2t_d, b1c_d, b2r_d, mT_d = ins
    out_d = outs[0]

    const = es.enter_context(tc.tile_pool(name="const", bufs=1))
    weights = es.enter_context(tc.tile_pool(name="weights", bufs=1))
    loads = es.enter_context(tc.tile_pool(name="loads", bufs=2))
    work = es.enter_context(tc.tile_pool(name="work", bufs=1))
    outp = es.enter_context(tc.tile_pool(name="outp", bufs=3))
    psA = es.enter_context(tc.tile_pool(name="psA", bufs=5, space="PSUM"))
    psB = es.enter_context(tc.tile_pool(name="psB", bufs=3, space="PSUM"))

    # ---- constants / replicated weights ----
    ident = const.tile([P, P], FP32)
    make_identity(nc, ident)
    ones_row = const.tile([1, P], FP32)
    nc.vector.memset(ones_row, 1.0)
    ones_col = const.tile([P, 1], FP32)
    nc.vector.memset(ones_col, 1.0)

    def load_elem(b, idx):
        cn = loads.tile([P, CT, D], FP32, tag="ctx_nat", name=f"ctx_nat{idx}")
        src_ap = ctx_d[b].rearrange("(t p) d -> p t d", p=P)
        half = CT // 2
        nc.sync.dma_start(cn[:, :half], src_ap[:, :half])
        nc.sync.dma_start(cn[:, half:], src_ap[:, half:])
        qn = loads.tile([P, D], FP32R, tag="qst_nat", name=f"qst_nat{idx}")
        nc.sync.dma_start(qn[:], qst_d[b])
        return cn, qn

    # elem-0 loads go before the big weight DMAs (single-shot only: with a
    # For_i timing loop the hoisted tile's slot would be recycled in-loop)
    pend = load_elem(0, 0) if reps == 1 else None

    vecsT = const.tile([P, DH, 3], FP32)  # [p, h, v]: wq/wc/wm at e=h*128+p
    nc.sync.dma_start(vecsT[:], vecsT_d.rearrange("(h p) v -> p h v", p=P))

    w1t = weights.tile([P, FT, F], FP32R)  # [fl, k, f'] = W1[f', k*128+fl]
    nc.sync.dma_start(w1t[:], w1t_d.rearrange("(k p) f -> p k f", p=P))
    w2t = weights.tile([P, FT, F], FP32R)  # [fl, k, f'] = W2[f', k*128+fl]
    nc.sync.dma_start(w2t[:], w2t_d.rearrange("(k p) f -> p k f", p=P))
    b1c = const.tile([P, FT], FP32)  # [p, mf] = b1[mf*128+p]
    nc.sync.dma_start(b1c[:], b1c_d)
    b2bc = const.tile([P, F], FP32)  # b2 broadcast to all partitions
    b2r_ap = b2r_d  # (1, F) in dram
    nc.gpsimd.dma_start(
        out=b2bc[:],
        in_=bass.AP(tensor=b2r_ap.tensor, offset=b2r_ap.offset, ap=[[0, P]] + b2r_ap.ap[1:]),
    )
    mT = const.tile([P, n_elems * CT], FP32)  # [p, b*8+t] = mask[b, t*128+p]
    nc.sync.dma_start(mT[:], mT_d)

    if reps > 1:
        es.enter_context(tc.For_i(0, reps, 1))

    def preamble(ctx_nat, qst_nat, idx):
        """PE transposes + DVE copies producing ctx^T, q^T, (q*wm)^T, qwq."""
        ctxT = work.tile([P, DH, C], FP32R, tag="ctxT", bufs=2, name=f"ctxT{idx}")
        for dh in range(DH):
            for g in range(2):  # two groups of 4 c-tiles -> one psum bank each
                pt = psA.tile([P, 512], FP32, tag="ps_mm", name=f"ptc{idx}{dh}{g}")
                for j in range(4):
                    t = g * 4 + j
                    nc.tensor.transpose(
                        pt[:, j * P:(j + 1) * P],
                        ctx_nat[:, t, dh * P:(dh + 1) * P],
                        ident[:],
                    )
                nc.vector.tensor_copy(ctxT[:, dh, g * 512:(g + 1) * 512], pt[:])

        qstT = work.tile([P, DH, P], FP32, tag="qstT", bufs=2, name=f"qstT{idx}")
        qmT = work.tile([P, DH, P], FP32R, tag="qmT", bufs=2, name=f"qmT{idx}")
        pq = psB.tile([P, 2 * P], FP32, tag="ps_small", name=f"pq{idx}")
        for dh in range(DH):
            nc.tensor.transpose(pq[:, dh * P:(dh + 1) * P],
                                _f(qst_nat[:, dh * P:(dh + 1) * P]), ident[:])
        nc.vector.tensor_copy(qstT[:].rearrange("p h q -> p (h q)"), pq[:])
        for dh in range(DH):
            nc.vector.tensor_scalar_mul(qmT[:, dh, :], qstT[:, dh, :], vecsT[:, dh, 2:3])

        # qwq term; cwc cancels in softmax-q entirely.  qwq needs no
        # per-sim-tile matmul: a broadcast tile is added during the fused
        # negate+max reduce, and exp reads the negated full logits.
        qwq = work.tile([1, P], FP32, tag="qwq", bufs=2, name=f"qwq{idx}")
        pw = psB.tile([1, P], FP32, tag="ps_small", name=f"pw{idx}")
        for dh in range(DH):
            nc.tensor.matmul(
                pw[:], vecsT[:, dh, 0:1], qstT[:, dh, :],
                start=(dh == 0), stop=(dh == DH - 1),
            )
        nc.vector.tensor_copy(qwq[:], pw[:])
        pqb = psA.tile([P, P], FP32, tag="ps_mm", name=f"pqb{idx}")
        nc.tensor.matmul(pqb[:], ones_row[:], qwq[:], start=True, stop=True)
        qwqbc = work.tile([P, P], FP32, tag="qwqbc", bufs=2, name=f"qwqbc{idx}")
        nc.vector.tensor_copy(qwqbc[:], pqb[:])
        return ctxT, qstT, qmT, qwqbc

    pre = None
    for b in range(n_elems):
        # ---- loads (elem b prefetched; prefetch b+1 now) ----
        if pend is None:
            pend = load_elem(b, b)
        ctx_nat, qst_nat = pend
        pend = load_elem(b + 1, b + 1) if b + 1 < n_elems else None
        if pre is None:
            pre = preamble(ctx_nat, qst_nat, b)
        ctxT, qstT, qmT, qwqbc = pre

        # ---- sim tiles + softmax over q (free dim) ----
        nmx = work.tile([P, CT], FP32, tag="nmx")     # negated row max per c-tile
        Pm = work.tile([P, CT, P], FP32, tag="Pm")    # softmax(sim), [c_l, t, q]
        sume = work.tile([P, CT], FP32, tag="sume")
        rs = work.tile([P, CT], FP32, tag="rs")
        for t in range(CT):
            ps = psB.tile([P, P], FP32, tag="ps_small")
            for dh in range(DH):
                nc.tensor.matmul(
                    ps[:], ctxT[:, dh, t * P:(t + 1) * P], qmT[:, dh, :],
                    start=(dh == 0), stop=(dh == DH - 1),
                )
            # scr = base + qwq (full logits); nmx = -max_q; exp with
            # fused row-sum accumulate
            scr = work.tile([P, P], FP32, tag="scr", bufs=2)
            nc.vector.tensor_add(scr[:], ps[:], qwqbc[:])
            nc.vector.reduce_max(nmx[:, t:t + 1], scr[:], axis=AX, negate=True)
            nc.scalar.activation(
                Pm[:, t, :], scr[:], AF.Exp, bias=nmx[:, t:t + 1],
                accum_out=sume[:, t:t + 1],
            )
            nc.vector.reciprocal(rs[:, t:t + 1], sume[:, t:t + 1])
            nc.vector.tensor_scalar_mul(Pm[:, t, :], Pm[:, t, :], rs[:, t:t + 1])
        PT = work.tile([P, C], FP32R, tag="PT")  # [q, c]
        for g in range(2):
            pt = psA.tile([P, 512], FP32, tag="ps_mm")
            for j in range(4):
                t = g * 4 + j
                nc.tensor.transpose(pt[:, j * P:(j + 1) * P], Pm[:, t, :], ident[:])
            nc.vector.tensor_copy(PT[:, g * 512:(g + 1) * 512], pt[:])

        # ---- q2c weights: softmax over all C of (max_q sim + cwc) ----
        pcw = psB.tile([P, CT], FP32, tag="ps_small")  # cwc as columns [c_l, t]
        for t in range(CT):
            for dh in range(DH):
                nc.tensor.matmul(
                    pcw[:, t:t + 1], _f(ctxT[:, dh, t * P:(t + 1) * P]),
                    vecsT[:, dh, 1:2],
                    start=(dh == 0), stop=(dh == DH - 1),
                )
        madj = work.tile([P, CT], FP32, tag="madj")  # m_c = cwc - nmx
        nc.vector.tensor_sub(madj[:], pcw[:], nmx[:])
        colmin = work.tile([P, 1], FP32, tag="colmin")
        nc.vector.reduce_max(colmin[:], madj[:], axis=AX, negate=True)
        pcm = psB.tile([1, P], FP32, tag="ps_small")
        nc.tensor.transpose(pcm[:], colmin[:], ident[:])
        minall = work.tile([1, 2], FP32, tag="minall")
        nc.vector.tensor_reduce(minall[:, 0:1], pcm[:], axis=AX, op=mybir.AluOpType.min)
        pmb = psB.tile([P, 1], FP32, tag="ps_small")
        nc.tensor.matmul(pmb[:], ones_row[:], minall[:, 0:1], start=True, stop=True)
        minb = work.tile([P, 1], FP32, tag="minb")
        nc.vector.tensor_copy(minb[:], pmb[:])
        wall = work.tile([P, CT], FP32, tag="wall")  # exp(m - Mglob)
        nc.scalar.activation(wall[:], madj[:], AF.Exp, bias=minb[:])

        # numerator columns (d,1) x2 and denominator
        pnum = [psB.tile([P, 1], FP32, tag="ps_small", name=f"pnum{dh}")
                for dh in range(DH)]
        for dh in range(DH):
            for t in range(CT):
                nc.tensor.matmul(
                    pnum[dh][:], ctx_nat[:, t, dh * P:(dh + 1) * P], wall[:, t:t + 1],
                    start=(t == 0), stop=(t == CT - 1),
                )
        pden = psB.tile([1, 1], FP32, tag="ps_small")
        for t in range(CT):
            nc.tensor.matmul(
                pden[:], wall[:, t:t + 1], ones_col[:],
                start=(t == 0), stop=(t == CT - 1),
            )
        rden = work.tile([1, 1], FP32, tag="rden")
        nc.vector.reciprocal(rden[:], pden[:])
        prb = psB.tile([P, 1], FP32, tag="ps_small")
        nc.tensor.matmul(prb[:], ones_row[:], rden[:], start=True, stop=True)
        rdenb = work.tile([P, 1], FP32, tag="rdenb")
        nc.vector.tensor_copy(rdenb[:], prb[:])
        q2c = work.tile([P, DH], FP32, tag="q2c")  # [d_l, dh]
        for dh in range(DH):
            nc.vector.tensor_mul(q2c[:, dh:dh + 1], pnum[dh][:], rdenb[:])

        # ---- att^T pieces ----
        c2qT = work.tile([P, DH, C], FP32R, tag="c2qT")
        for dh in range(DH):
            for g in range(NH):
                pc2 = psA.tile([P, 512], FP32, tag="ps_mm")
                nc.tensor.matmul(
                    pc2[:], qst_nat[:, dh * P:(dh + 1) * P],
                    PT[:, g * 512:(g + 1) * 512],
                    start=True, stop=True,
                )
                nc.vector.tensor_copy(c2qT[:, dh, g * 512:(g + 1) * 512], pc2[:])
        cxc = work.tile([P, DH, C], FP32R, tag="cxc")
        cxq = work.tile([P, DH, C], FP32R, tag="cxq")
        for dh in range(DH):
            nc.vector.tensor_mul(cxc[:, dh, :], _f(ctxT[:, dh, :]), _f(c2qT[:, dh, :]))
            nc.vector.tensor_scalar_mul(cxq[:, dh, :], _f(ctxT[:, dh, :]), q2c[:, dh:dh + 1])

        att_pieces = [ctxT, c2qT, cxc, cxq]  # k-tile = att_pieces[k//2][:, k%2, :]

        # ---- layer 1: h1^T[f', c] = W1 @ att^T + b1 ----
        h1T = work.tile([P, FT, C], FP32R, tag="h1T")
        for mf in range(FT):
            for g in range(NH):
                ph = psA.tile([P, 512], FP32, tag="ps_mm")
                for k in range(FT):
                    rhs = att_pieces[k // 2][:, k % 2, g * 512:(g + 1) * 512]
                    nc.tensor.matmul(
                        ph[:], w1t[:, k, mf * P:(mf + 1) * P], rhs,
                        start=(k == 0), stop=(k == FT - 1),
                    )
                # alternate PSUM eviction between ACT and DVE to release
                # psA slots faster
                if (mf * NH + g) % 2 == 0:
                    nc.scalar.add(h1T[:, mf, g * 512:(g + 1) * 512], ph[:],
                                  b1c[:, mf:mf + 1])
                else:
                    nc.vector.tensor_scalar_add(h1T[:, mf, g * 512:(g + 1) * 512],
                                                ph[:], b1c[:, mf:mf + 1])

        # next elem's transpose preamble: PE runs it here so its DVE copies
        # overlap layer-2 matmuls instead of stalling at the elem boundary
        pre = preamble(pend[0], pend[1], b + 1) if pend is not None else None

        # ---- layer 2 (natural layout) + bias + mask + relu + store ----
        for ct in range(CT):
            osb = outp.tile([P, F], FP32, tag="osb")
            for fh in range(FH):
                p2 = psA.tile([P, 512], FP32, tag="ps_mm")
                for k in range(FT):
                    nc.tensor.matmul(
                        p2[:], h1T[:, k, ct * P:(ct + 1) * P],
                        w2t[:, k, fh * 512:(fh + 1) * 512],
                        start=(k == 0), stop=(k == FT - 1),
                    )
                tmp = outp.tile([P, 512], FP32, tag="tmp")
                nc.vector.tensor_add(tmp[:], p2[:], b2bc[:, fh * 512:(fh + 1) * 512])
                nc.scalar.activation(
                    osb[:, fh * 512:(fh + 1) * 512], tmp[:], AF.Relu,
                    scale=mT[:, b * CT + ct:b * CT + ct + 1],
                )
                nc.sync.dma_start(
                    out_d[b, ct * P:(ct + 1) * P, fh * 512:(fh + 1) * 512],
                    osb[:, fh * 512:(fh + 1) * 512])


_NC_CACHE = {}


def _build_nc(n_elems=BPC, reps=1):
    key = (n_elems, reps)
    if key in _NC_CACHE:
        return _NC_CACHE[key]
    nc = bacc.Bacc("TRN2", target_bir_lowering=False, debug=False, num_devices=NCORES)
    ins = [
        nc.dram_tensor("ctx", (n_elems, C, D), FP32, kind="ExternalInput").ap(),
        nc.dram_tensor("qst", (n_elems, Q, D), FP32R, kind="ExternalInput").ap(),
        nc.dram_tensor("vecsT", (D, 3), FP32, kind="ExternalInput").ap(),
        nc.dram_tensor("w1t", (F, F), FP32R, kind="ExternalInput").ap(),
        nc.dram_tensor("w2t", (F, F), FP32R, kind="ExternalInput").ap(),
        nc.dram_tensor("b1c", (P, FT), FP32, kind="ExternalInput").ap(),
        nc.dram_tensor("b2r", (1, F), FP32, kind="ExternalInput").ap(),
        nc.dram_tensor("mT", (P, n_elems * CT), FP32, kind="ExternalInput").ap(),
    ]
    outs = [nc.dram_tensor("out", (n_elems, C, F), FP32, kind="ExternalOutput").ap()]
    from contextlib import ExitStack
    with tile.TileContext(nc) as tc, ExitStack() as es:
        _build_body(es, tc, outs, ins, n_elems=n_elems, reps=reps)
    nc.compile()
    _NC_CACHE[key] = (nc, ins, outs)
    return _NC_CACHE[key]


def _host_prep(context, question, context_mask, w_question, w_context, w_multiple,
               W1, b1, W2, b2):
    """Build the 8 per-core input maps from full inputs."""
    context = np.asarray(context, np.float32)
    question = np.asarray(question, np.float32)
    maskf = np.asarray(context_mask).astype(np.float32)
    vecsT = np.ascontiguousarray(
        np.stack([w_question, w_context, w_multiple]).T.astype(np.float32))  # (D,3)
    w1t = np.ascontiguousarray(np.asarray(W1, np.float32).T)  # [f, f'] = W1[f', f]
    w2t = np.ascontiguousarray(np.asarray(W2, np.float32).T)
    b1c = np.ascontiguousarray(np.asarray(b1, np.float32).reshape(FT, P).T)  # (128, 8)
    b2r = np.asarray(b2, np.float32).reshape(1, F)
    in_maps = []
    for i in range(NCORES):
        sl = slice(BPC * i, BPC * (i + 1))
        mTc = np.ascontiguousarray(
            maskf[sl].reshape(BPC, CT, P).transpose(2, 0, 1).reshape(P, BPC * CT))
        in_maps.append({
            "ctx": np.ascontiguousarray(context[sl]),
            "qst": np.ascontiguousarray(question[sl]),
            "vecsT": vecsT,
            "w1t": w1t,
            "w2t": w2t,
            "b1c": b1c,
            "b2r": b2r,
            "mT": mTc,
        })
    return in_maps


def kernel(context, question, context_mask, w_question, w_context, w_multiple,
           W1, b1, W2, b2):
    nc, _, _ = _build_nc()
    in_maps = _host_prep(context, question, context_mask, w_question, w_context,
                         w_multiple, W1, b1, W2, b2)
    res = run_bass_kernel_spmd(nc, in_maps, list(range(NCORES))).results
    out = np.concatenate([res[i]["out"] for i in range(NCORES)], axis=0)
    return out


# revision 34
# speedup vs baseline: 1.0218x; 1.0004x over previous
"""Trainium2 Bass kernel for BaseBidirectionalAttention.

Problem shapes (hardcoded): B=32, C=1024, Q=128, D=256, F=4D=1024.
Sharding: data-parallel over batch across 8 cores (4 batch elems/core);
weights replicated.

Per-core program (per batch elem):
  sim(C,Q)   = ctx @ (q*wm).T (+qwq via broadcast add)       (PE, c on partitions)
  P          = softmax_q(sim)  [cwc term cancels here]       (DVE/ACT, free-dim)
  c2q^T(D,C) = question.T @ P.T                              (PE, via P transpose)
  q2c(D)     = softmax_c(max_q sim + cwc) @ ctx              (PE + transpose trick)
  att^T      = [ctx^T; c2q^T; ctx^T*c2q^T; ctx^T*q2c]        (f on partitions)
  h1^T(F,C)  = W1 @ att^T + b1                               (PE, lhsT=W1T tiles)
  out(C,F)   = relu((h1 @ W2.T + b2)) * mask                 (PE, natural layout)

The intermediate mask multiply of the reference is provably redundant (row
masks propagate row-wise through the linears), so masking happens once at the
end.  The heavy matmuls (sim, c2q, both linears) run in float32r: full-rate PE
streaming (~1 col/cycle vs 4 for fp32) at slightly reduced mantissa precision;
rel err vs the fp32 reference is ~5.7e-4.  Softmax statistics, exp, and all
reductions stay fp32.
"""

import sys

if "/opt/trn_rl_repo" not in sys.path:
    sys.path.insert(0, "/opt/trn_rl_repo")

import numpy as np

import concourse.bass as bass
import concourse.mybir as mybir
import concourse.tile as tile
from concourse import bacc
from concourse.bass_utils import run_bass_kernel_spmd
from concourse.masks import make_identity

B, C, Q, D = 32, 1024, 128, 256
F = 4 * D
NCORES = 8
BPC = B // NCORES  # batch elems per core
P = 128
CT = C // P   # 8 c-tiles
FT = F // P   # 8 f-tiles
DH = D // P   # 2 halves of D
NH = C // 512  # 2 c-chunks of 512
FH = F // 512  # 2 f'-chunks of 512

FP32 = mybir.dt.float32
FP32R = mybir.dt.float32r
AX = mybir.AxisListType.X
AF = mybir.ActivationFunctionType


def _f(ap):
    """fp32 view of a float32r AP (same bits) for DVE/fp32-matmul reads."""
    return ap.bitcast(FP32)


def _build_body(es, tc, outs, ins, n_elems=BPC, reps=1):
    nc = tc.nc
    ctx_d, qst_d, vecsT_d, w1t_d, w2t_d, b1c_d, b2r_d, mT_d = ins
    out_d = outs[0]

    const = es.enter_context(tc.tile_pool(name="const", bufs=1))
    weights = es.enter_context(tc.tile_pool(name="weights", bufs=1))
    loads = es.enter_context(tc.tile_pool(name="loads", bufs=2))
    work = es.enter_context(tc.tile_pool(name="work", bufs=1))
    outp = es.enter_context(tc.tile_pool(name="outp", bufs=3))
    psA = es.enter_context(tc.tile_pool(name="psA", bufs=5, space="PSUM"))
    psB = es.enter_context(tc.tile_pool(name="psB", bufs=3, space="PSUM"))

    # ---- constants / replicated weights ----
    ident = const.tile([P, P], FP32)
    make_identity(nc, ident)
    ones_row = const.tile([1, P], FP32)
    nc.vector.memset(ones_row, 1.0)
    ones_col = const.tile([P, 1], FP32)
    nc.vector.memset(ones_col, 1.0)

    def load_elem(b, idx):
        cn = loads.tile([P, CT, D], FP32, tag="ctx_nat", name=f"ctx_nat{idx}")
        src_ap = ctx_d[b].rearrange("(t p) d -> p t d", p=P)
        half = CT // 2
        nc.sync.dma_start(cn[:, :half], src_ap[:, :half])
        nc.sync.dma_start(cn[:, half:], src_ap[:, half:])
        qn = loads.tile([P, D], FP32R, tag="qst_nat", name=f"qst_nat{idx}")
        nc.sync.dma_start(qn[:], qst_d[b])
        return cn, qn

    # elem-0 loads go before the big weight DMAs (single-shot only: with a
    # For_i timing loop the hoisted tile's slot would be recycled in-loop)
    pend = load_elem(0, 0) if reps == 1 else None

    vecsT = const.tile([P, DH, 3], FP32)  # [p, h, v]: wq/wc/wm at e=h*128+p
    nc.sync.dma_start(vecsT[:], vecsT_d.rearrange("(h p) v -> p h v", p=P))

    w1t = weights.tile([P, FT, F], FP32R)  # [fl, k, f'] = W1[f', k*128+fl]
    nc.sync.dma_start(w1t[:], w1t_d.rearrange("(k p) f -> p k f", p=P))
    w2t = weights.tile([P, FT, F], FP32R)  # [fl, k, f'] = W2[f', k*128+fl]
    nc.sync.dma_start(w2t[:], w2t_d.rearrange("(k p) f -> p k f", p=P))
    b1c = const.tile([P, FT], FP32)  # [p, mf] = b1[mf*128+p]
    nc.sync.dma_start(b1c[:], b1c_d)
    b2bc = const.tile([P, F], FP32)  # b2 broadcast to all partitions
    b2r_ap = b2r_d  # (1, F) in dram
    nc.gpsimd.dma_start(
        out=b2bc[:],
        in_=bass.AP(tensor=b2r_ap.tensor, offset=b2r_ap.offset, ap=[[0, P]] + b2r_ap.ap[1:]),
    )
    mT = const.tile([P, n_elems * CT], FP32)  # [p, b*8+t] = mask[b, t*128+p]
    nc.sync.dma_start(mT[:], mT_d)

    if reps > 1:
        es.enter_context(tc.For_i(0, reps, 1))

    def preamble(ctx_nat, qst_nat, idx):
        """PE transposes + DVE copies producing ctx^T, q^T, (q*wm)^T, qwq."""
        ctxT = work.tile([P, DH, C], FP32R, tag="ctxT", bufs=2, name=f"ctxT{idx}")
        for dh in range(DH):
            for g in range(2):  # two groups of 4 c-tiles -> one psum bank each
                pt = psA.tile([P, 512], FP32, tag="ps_mm", name=f"ptc{idx}{dh}{g}")
                for j in range(4):
                    t = g * 4 + j
                    nc.tensor.transpose(
                        pt[:, j * P:(j + 1) * P],
                        ctx_nat[:, t, dh * P:(dh + 1) * P],
                        ident[:],
                    )
                nc.vector.tensor_copy(ctxT[:, dh, g * 512:(g + 1) * 512], pt[:])

        qstT = work.tile([P, DH, P], FP32, tag="qstT", bufs=2, name=f"qstT{idx}")
        qmT = work.tile([P, DH, P], FP32R, tag="qmT", bufs=2, name=f"qmT{idx}")
        pq = psB.tile([P, 2 * P], FP32, tag="ps_small", name=f"pq{idx}")
        for dh in range(DH):
            nc.tensor.transpose(pq[:, dh * P:(dh + 1) * P],
                                _f(qst_nat[:, dh * P:(dh + 1) * P]), ident[:])
        nc.vector.tensor_copy(qstT[:].rearrange("p h q -> p (h q)"), pq[:])
        for dh in range(DH):
            nc.vector.tensor_scalar_mul(qmT[:, dh, :], qstT[:, dh, :], vecsT[:, dh, 2:3])

        # qwq term; cwc cancels in softmax-q entirely.  qwq needs no
        # per-sim-tile matmul: a broadcast tile is added during the fused
        # negate+max reduce, and exp reads the negated full logits.
        qwq = work.tile([1, P], FP32, tag="qwq", bufs=2, name=f"qwq{idx}")
        pw = psB.tile([1, P], FP32, tag="ps_small", name=f"pw{idx}")
        for dh in range(DH):
            nc.tensor.matmul(
                pw[:], vecsT[:, dh, 0:1], qstT[:, dh, :],
                start=(dh == 0), stop=(dh == DH - 1),
            )
        nc.vector.tensor_copy(qwq[:], pw[:])
        pqb = psA.tile([P, P], FP32, tag="ps_mm", name=f"pqb{idx}")
        nc.tensor.matmul(pqb[:], ones_row[:], qwq[:], start=True, stop=True)
        qwqbc = work.tile([P, P], FP32, tag="qwqbc", bufs=2, name=f"qwqbc{idx}")
        nc.vector.tensor_copy(qwqbc[:], pqb[:])
        return ctxT, qstT, qmT, qwqbc

    pre = None
    for b in range(n_elems):
        # ---- loads (elem b prefetched; prefetch b+1 now) ----
        if pend is None:
            pend = load_elem(b, b)
        ctx_nat, qst_nat = pend
        pend = load_elem(b + 1, b + 1) if b + 1 < n_elems else None
        if pre is None:
            pre = preamble(ctx_nat, qst_nat, b)
        ctxT, qstT, qmT, qwqbc = pre

        # ---- sim tiles + softmax over q (free dim) ----
        nmx = work.tile([P, CT], FP32, tag="nmx")     # negated row max per c-tile
        Pm = work.tile([P, CT, P], FP32, tag="Pm")    # softmax(sim), [c_l, t, q]
        sume = work.tile([P, CT], FP32, tag="sume")
        rs = work.tile([P, CT], FP32, tag="rs")
        for t in range(CT):
            ps = psB.tile([P, P], FP32, tag="ps_small")
            for dh in range(DH):
                nc.tensor.matmul(
                    ps[:], ctxT[:, dh, t * P:(t + 1) * P], qmT[:, dh, :],
                    start=(dh == 0), stop=(dh == DH - 1),
                )
            # scr = base + qwq (full logits); nmx = -max_q; exp with
            # fused row-sum accumulate
            scr = work.tile([P, P], FP32, tag="scr", bufs=2)
            nc.vector.tensor_add(scr[:], ps[:], qwqbc[:])
            nc.vector.reduce_max(nmx[:, t:t + 1], scr[:], axis=AX, negate=True)
            nc.scalar.activation(
                Pm[:, t, :], scr[:], AF.Exp, bias=nmx[:, t:t + 1],
                accum_out=sume[:, t:t + 1],
            )
            nc.vector.reciprocal(rs[:, t:t + 1], sume[:, t:t + 1])
            nc.vector.tensor_scalar_mul(Pm[:, t, :], Pm[:, t, :], rs[:, t:t + 1])
        PT = work.tile([P, C], FP32R, tag="PT")  # [q, c]
        for g in range(2):
            pt = psA.tile([P, 512], FP32, tag="ps_mm")
            for j in range(4):
                t = g * 4 + j
                nc.tensor.transpose(pt[:, j * P:(j + 1) * P], Pm[:, t, :], ident[:])
            nc.vector.tensor_copy(PT[:, g * 512:(g + 1) * 512], pt[:])

        # ---- q2c weights: softmax over all C of (max_q sim + cwc) ----
        pcw = psB.tile([P, CT], FP32, tag="ps_small")  # cwc as columns [c_l, t]
        for t in range(CT):
            for dh in range(DH):
                nc.tensor.matmul(
                    pcw[:, t:t + 1], _f(ctxT[:, dh, t * P:(t + 1) * P]),
                    vecsT[:, dh, 1:2],
                    start=(dh == 0), stop=(dh == DH - 1),
                )
        madj = work.tile([P, CT], FP32, tag="madj")  # m_c = cwc - nmx
        nc.vector.tensor_sub(madj[:], pcw[:], nmx[:])
        colmin = work.tile([P, 1], FP32, tag="colmin")
        nc.vector.reduce_max(colmin[:], madj[:], axis=AX, negate=True)
        pcm = psB.tile([1, P], FP32, tag="ps_small")
        nc.tensor.transpose(pcm[:], colmin[:], ident[:])
        minall = work.tile([1, 2], FP32, tag="minall")
        nc.vector.tensor_reduce(minall[:, 0:1], pcm[:], axis=AX, op=mybir.AluOpType.min)
        pmb = psB.tile([P, 1], FP32, tag="ps_small")
        nc.tensor.matmul(pmb[:], ones_row[:], minall[:, 0:1], start=True, stop=True)
        minb = work.tile([P, 1], FP32, tag="minb")
        nc.vector.tensor_copy(minb[:], pmb[:])
        wall = work.tile([P, CT], FP32, tag="wall")  # exp(m - Mglob)
        nc.scalar.activation(wall[:], madj[:], AF.Exp, bias=minb[:])

        # numerator columns (d,1) x2 and denominator
        pnum = [psB.tile([P, 1], FP32, tag="ps_small", name=f"pnum{dh}")
                for dh in range(DH)]
        for dh in range(DH):
            for t in range(CT):
                nc.tensor.matmul(
                    pnum[dh][:], ctx_nat[:, t, dh * P:(dh + 1) * P], wall[:, t:t + 1],
                    start=(t == 0), stop=(t == CT - 1),
                )
        pden = psB.tile([1, 1], FP32, tag="ps_small")
        for t in range(CT):
            nc.tensor.matmul(
                pden[:], wall[:, t:t + 1], ones_col[:],
                start=(t == 0), stop=(t == CT - 1),
            )
        rden = work.tile([1, 1], FP32, tag="rden")
        nc.vector.reciprocal(rden[:], pden[:])
        prb = psB.tile([P, 1], FP32, tag="ps_small")
        nc.tensor.matmul(prb[:], ones_row[:], rden[:], start=True, stop=True)
        rdenb = work.tile([P, 1], FP32, tag="rdenb")
        nc.vector.tensor_copy(rdenb[:], prb[:])
        q2c = work.tile([P, DH], FP32, tag="q2c")  # [d_l, dh]
        for dh in range(DH):
            nc.vector.tensor_mul(q2c[:, dh:dh + 1], pnum[dh][:], rdenb[:])

        # ---- att^T pieces ----
        c2qT = work.tile([P, DH, C], FP32R, tag="c2qT")
        for dh in range(DH):
            for g in range(NH):
                pc2 = psA.tile([P, 512], FP32, tag="ps_mm")
                nc.tensor.matmul(
                    pc2[:], qst_nat[:, dh * P:(dh + 1) * P],
                    PT[:, g * 512:(g + 1) * 512],
                    start=True, stop=True,
                )
                nc.vector.tensor_copy(c2qT[:, dh, g * 512:(g + 1) * 512], pc2[:])
        cxc = work.tile([P, DH, C], FP32R, tag="cxc")
        cxq = work.tile([P, DH, C], FP32R, tag="cxq")
        for dh in range(DH):
            nc.vector.tensor_mul(cxc[:, dh, :], _f(ctxT[:, dh, :]), _f(c2qT[:, dh, :]))
            nc.vector.tensor_scalar_mul(cxq[:, dh, :], _f(ctxT[:, dh, :]), q2c[:, dh:dh + 1])

        att_pieces = [ctxT, c2qT, cxc, cxq]  # k-tile = att_pieces[k//2][:, k%2, :]

        # ---- layer 1: h1^T[f', c] = W1 @ att^T + b1 ----
        h1T = work.tile([P, FT, C], FP32R, tag="h1T")
        for mf in range(FT):
            for g in range(NH):
                ph = psA.tile([P, 512], FP32, tag="ps_mm")
                for k in range(FT):
                    rhs = att_pieces[k // 2][:, k % 2, g * 512:(g + 1) * 512]
                    nc.tensor.matmul(
                        ph[:], w1t[:, k, mf * P:(mf + 1) * P], rhs,
                        start=(k == 0), stop=(k == FT - 1),
                    )
                # alternate PSUM eviction between ACT and DVE to release
                # psA slots faster
                if (mf * NH + g) % 2 == 0:
                    nc.scalar.add(h1T[:, mf, g * 512:(g + 1) * 512], ph[:],
                                  b1c[:, mf:mf + 1])
                else:
                    nc.vector.tensor_scalar_add(h1T[:, mf, g * 512:(g + 1) * 512],
                                                ph[:], b1c[:, mf:mf + 1])

        # next elem's transpose preamble: PE runs it here so its DVE copies
        # overlap layer-2 matmuls instead of stalling at the elem boundary
        pre = preamble(pend[0], pend[1], b + 1) if pend is not None else None

        # ---- layer 2 (natural layout) + bias + mask + relu + store ----
        for ct in range(CT):
            osb = outp.tile([P, F], FP32, tag="osb")
            for fh in range(FH):
                p2 = psA.tile([P, 512], FP32, tag="ps_mm")
                for k in range(FT):
                    nc.tensor.matmul(
                        p2[:], h1T[:, k, ct * P:(ct + 1) * P],
                        w2t[:, k, fh * 512:(fh + 1) * 512],
                        start=(k == 0), stop=(k == FT - 1),
                    )
                tmp = outp.tile([P, 512], FP32, tag="tmp")
                nc.vector.tensor_add(tmp[:], p2[:], b2bc[:, fh * 512:(fh + 1) * 512])
                nc.scalar.activation(
                    osb[:, fh * 512:(fh + 1) * 512], tmp[:], AF.Relu,
                    scale=mT[:, b * CT + ct:b * CT + ct + 1],
                )
                nc.sync.dma_start(
                    out_d[b, ct * P:(ct + 1) * P, fh * 512:(fh + 1) * 512],
                    osb[:, fh * 512:(fh + 1) * 512])


_NC_CACHE = {}


def _build_nc(n_elems=BPC, reps=1):
    key = (n_elems, reps)
    if key in _NC_CACHE:
        return _NC_CACHE[key]
    nc = bacc.Bacc("TRN2", target_bir_lowering=False, debug=False, num_devices=NCORES)
    ins = [
        nc.dram_tensor("ctx", (n_elems, C, D), FP32, kind="ExternalInput").ap(),
        nc.dram_tensor("qst", (n_elems, Q, D), FP32R, kind="ExternalInput").ap(),
        nc.dram_tensor("vecsT", (D, 3), FP32, kind="ExternalInput").ap(),
        nc.dram_tensor("w1t", (F, F), FP32R, kind="ExternalInput").ap(),
        nc.dram_tensor("w2t", (F, F), FP32R, kind="ExternalInput").ap(),
        nc.dram_tensor("b1c", (P, FT), FP32, kind="ExternalInput").ap(),
        nc.dram_tensor("b2r", (1, F), FP32, kind="ExternalInput").ap(),
        nc.dram_tensor("mT", (P, n_elems * CT), FP32, kind="ExternalInput").ap(),
    ]
    outs = [nc.dram_tensor("out", (n_elems, C, F), FP32, kind="ExternalOutput").ap()]
    from contextlib import ExitStack
    with tile.TileContext(nc) as tc, ExitStack() as es:
        _build_body(es, tc, outs, ins, n_elems=n_elems, reps=reps)
    nc.compile()
    _NC_CACHE[key] = (nc, ins, outs)
    return _NC_CACHE[key]


def _host_prep(context, question, context_mask, w_question, w_context, w_multiple,
               W1, b1, W2, b2):
    """Build the 8 per-core input maps from full inputs."""
    context = np.asarray(context, np.float32)
    question = np.asarray(question, np.float32)
    maskf = np.asarray(context_mask).astype(np.float32)
    vecsT = np.ascontiguousarray(
        np.stack([w_question, w_context, w_multiple]).T.astype(np.float32))  # (D,3)
    w1t = np.ascontiguousarray(np.asarray(W1, np.float32).T)  # [f, f'] = W1[f', f]
    w2t = np.ascontiguousarray(np.asarray(W2, np.float32).T)
    b1c = np.ascontiguousarray(np.asarray(b1, np.float32).reshape(FT, P).T)  # (128, 8)
    b2r = np.asarray(b2, np.float32).reshape(1, F)
    in_maps = []
    for i in range(NCORES):
        sl = slice(BPC * i, BPC * (i + 1))
        mTc = np.ascontiguousarray(
            maskf[sl].reshape(BPC, CT, P).transpose(2, 0, 1).reshape(P, BPC * CT))
        in_maps.append({
            "ctx": np.ascontiguousarray(context[sl]),
            "qst": np.ascontiguousarray(question[sl]),
            "vecsT": vecsT,
            "w1t": w1t,
            "w2t": w2t,
            "b1c": b1c,
            "b2r": b2r,
            "mT": mTc,
        })
    return in_maps


def kernel(context, question, context_mask, w_question, w_context, w_multiple,
           W1, b1, W2, b2):
    nc, _, _ = _build_nc()
    in_maps = _host_prep(context, question, context_mask, w_question, w_context,
                         w_multiple, W1, b1, W2, b2)
    res = run_bass_kernel_spmd(nc, in_maps, list(range(NCORES))).results
    out = np.concatenate([res[i]["out"] for i in range(NCORES)], axis=0)
    return out
